# revision 7
# baseline (speedup 1.0000x reference)
"""Trainium2 Bass kernel for int8-dequant Linear: out = x @ (W_q * scaler)^T.

Full shapes: x [4, 2048, 4096] f32, weight_q [4096, 4096] int8,
weight_scaler [4096] f32 -> out [4, 2048, 4096] f32.

Sharding: data-parallel over tokens (8192 tokens -> 1024 per core);
weight_q/scaler replicated. Each core computes out.T for its token
shard with out-channels on PSUM partitions; the per-channel scaler is
applied as a per-partition scalar multiply on PSUM eviction.

Matmul dtype: float32r (TF32-like fast fp32 mode, 1 cyc/row at free
dim >= 256; measured rel-err ~2e-4 on K=4096 accumulation).
Fallback MODE "bf16x2": W exact in bf16 (int8-valued), x split into
bf16 hi+lo, two accumulation passes (rel err ~5e-6, 2x matmul work).
"""
import sys

sys.path.insert(0, "/opt/trn_rl_repo")

import numpy as np

import concourse.bacc as bacc
import concourse.mybir as mybir
import concourse.tile as tile
from concourse.bass_utils import run_bass_kernel_spmd

N_CORES = 8
P = 128
IN_F = 4096
OUT_F = 4096
TOKENS = 4 * 2048
T_SHARD = TOKENS // N_CORES          # 1024 tokens per core
KT = IN_F // P                       # 32 k-tiles
MT = OUT_F // P                      # 32 m-tiles (out-channel tiles)
N_FREE = 512                         # moving free dim per matmul (1 PSUM bank)
NT = T_SHARD // N_FREE               # 2 n-tiles

MODE = "f32r"                        # "f32r" | "bf16x2"

_cache = {}


def _build(mode):
    f32 = mybir.dt.float32
    mm_dt = mybir.dt.float32r if mode == "f32r" else mybir.dt.bfloat16
    n_pass = 1 if mode == "f32r" else 2

    nc = bacc.Bacc(None, target_bir_lowering=False, debug=False)

    # DRAM parameters (per-core shapes)
    d_x = [
        nc.declare_dram_parameter(f"xq{i}", [IN_F, T_SHARD], mm_dt, isOutput=False)
        for i in range(n_pass)
    ]
    d_w = nc.declare_dram_parameter("wq4", [MT, P, KT, P], mm_dt, isOutput=False)
    d_s = nc.declare_dram_parameter("scal", [P, MT], f32, isOutput=False)
    d_o = nc.declare_dram_parameter("outT", [MT, P, T_SHARD], f32, isOutput=True)

    with tile.TileContext(nc) as tc:
        with (
            tc.tile_pool(name="xp", bufs=KT * n_pass) as xp,
            tc.tile_pool(name="wp", bufs=3) as wp,
            tc.tile_pool(name="op", bufs=4) as op,
            tc.tile_pool(name="cp", bufs=1) as cp,
            tc.tile_pool(name="ps", bufs=8, space="PSUM") as ps,
        ):
            scal = cp.tile([P, MT], f32)
            nc.sync.dma_start(scal[:], d_s.ap())

            # x: one SBUF tile per (pass, k-tile); DMA is emitted lazily at
            # first use so the first weight tile + first x slice land before
            # the bulk of the 16MB x load and the PE starts early.
            xt = [None] * (KT * n_pass)

            def get_x(i):
                if xt[i] is None:
                    ip, k = divmod(i, KT)
                    t = xp.tile([P, T_SHARD], mm_dt, tag="xt", name=f"xt_{i}")
                    nc.sync.dma_start(t[:], d_x[ip].ap()[k * P:(k + 1) * P, :])
                    xt[i] = t
                return xt[i]

            nk = KT * n_pass
            for mo in range(MT):
                wt = wp.tile([P, KT, P], mm_dt, tag="wt")
                nc.sync.dma_start(wt[:], d_w.ap()[mo])
                # n innermost: each weight tile is loaded into the PE once
                # and reused for all NT moving slices before switching.
                psums = [
                    ps.tile([P, N_FREE], f32, tag="psum", name=f"psum_{mo}_{n}")
                    for n in range(NT)
                ]
                for i in range(nk):
                    ip, k = divmod(i, KT)
                    xti = get_x(i)
                    for n in range(NT):
                        nc.tensor.matmul(
                            psums[n][:],
                            wt[:, k, :],
                            xti[:, n * N_FREE:(n + 1) * N_FREE],
                            start=(i == 0),
                            stop=(i == nk - 1),
                        )
                for n in range(NT):
                    osb = op.tile([P, N_FREE], f32, tag="osb")
                    nc.vector.tensor_scalar_mul(osb[:], psums[n][:], scal[:, mo:mo + 1])
                    nc.sync.dma_start(
                        d_o.ap()[mo, :, n * N_FREE:(n + 1) * N_FREE], osb[:]
                    )

    nc.compile()
    return nc


def _prep_inputs(x, weight_q, weight_scaler, mode):
    """Host-side shard + layout. Returns in_maps (list of dicts, one per core)."""
    xf = np.asarray(x, dtype=np.float32).reshape(TOKENS, IN_F)
    wq = np.asarray(weight_q)
    sc = np.asarray(weight_scaler, dtype=np.float32)

    # W tiles: w4[mo, p_in, ko, oc] = W[mo*128+oc, ko*128+p_in]
    # (matches the SBUF lhsT tile AP [P, KT, P] exactly)
    if mode == "f32r":
        w4 = np.ascontiguousarray(
            wq.reshape(MT, P, KT, P).transpose(0, 3, 2, 1), dtype=np.float32
        )
    else:
        import ml_dtypes

        w4 = np.ascontiguousarray(
            wq.reshape(MT, P, KT, P).transpose(0, 3, 2, 1)
        ).astype(ml_dtypes.bfloat16)

    scal = np.ascontiguousarray(sc.reshape(MT, P).T)  # [P, MT]

    in_maps = []
    for c in range(N_CORES):
        xs = xf[c * T_SHARD:(c + 1) * T_SHARD, :]      # [T_SHARD, IN_F]
        xsT = np.ascontiguousarray(xs.T)                # [IN_F, T_SHARD] f32
        m = {"wq4": w4, "scal": scal}
        if mode == "f32r":
            m["xq0"] = xsT
        else:
            import ml_dtypes

            hi = xsT.astype(ml_dtypes.bfloat16)
            lo = (xsT - hi.astype(np.float32)).astype(ml_dtypes.bfloat16)
            m["xq0"] = hi
            m["xq1"] = lo
        in_maps.append(m)
    return in_maps


def _gather(results):
    """Per-core outT [MT, P, T_SHARD] -> full out [4, 2048, OUT_F] f32."""
    parts = []
    for c in range(N_CORES):
        ot = results[c]["outT"]                   # [MT, P, T_SHARD]
        parts.append(ot.reshape(OUT_F, T_SHARD).T)  # [T_SHARD, OUT_F]
    out = np.concatenate(parts, axis=0)           # [TOKENS, OUT_F]
    return np.ascontiguousarray(out.reshape(4, 2048, OUT_F), dtype=np.float32)


def _run(inputs, trace=False, mode=None):
    mode = mode or MODE
    if mode not in _cache:
        _cache[mode] = _build(mode)
    nc = _cache[mode]
    in_maps = _prep_inputs(inputs["x"], inputs["weight_q"], inputs["weight_scaler"], mode)
    res = run_bass_kernel_spmd(nc, in_maps, list(range(N_CORES)), trace=trace)
    return _gather(res.results), res


def kernel(**inputs):
    out, _ = _run(inputs, trace=False)
    return out
